# revision 49
# baseline (speedup 1.0000x reference)
"""Trainium2 Bass kernel for nn_DiagonalMicroAttention (3x3 neighborhood sparse attention).

Sharding: 8 cores x 7 query rows (both batches per core). Channel-major layout.
Per core: Q/K/V via 1x1-conv matmuls, 9-offset sparse attention with
partition-stacked softmax rows p = 9h+o per batch, rank-1 additive boundary
masking folded into the dots PSUM accumulation, softmax normalization and the
asymmetry gate folded into one per-(b,h) scale, e2 broadcast to channels via
selector matmuls, and the output projection PSUM-accumulated over the 9
offsets.

Schedule: batch-0 and batch-1 streams are software-pipelined; batch-0's
elementwise work runs on DVE while Pool produces batch-1's q*k products; the
e-broadcast PSUM tiles are evacuated in pairs on ACT (one offset per trio is
consumed from PSUM directly by DVE). A PE warm-up chain plus wait-queue
blockers hold the first real matmuls past the p-state ramp so every matmul is
costed at full clock. Input DMA is split 5 ways in criticality order. The
gelu/sigmoid are computed in tanh form so one activation LUT set serves the
whole kernel.
"""
import numpy as np

import concourse.bass as bass
import concourse.tile as tile
from concourse import bacc, mybir
from concourse.bass_utils import run_bass_kernel_spmd

F32 = mybir.dt.float32
BF16 = mybir.dt.bfloat16
AF = mybir.ActivationFunctionType
ALU = mybir.AluOpType

B, C, H, W, HEADS = 2, 128, 56, 56, 4
DH = C // HEADS
SCALE = float(DH) ** -0.5
NCORES = 8
RQ, RH, WP = 7, 9, 58
NQ = RQ * W          # 392 queries per batch per core
BIG = 30000.0
S36 = 36             # stacked softmax partitions per batch: p = 9*h + o

# pkb1 (bf16, 128 x 2112): inputs + projection weights
XS, WQ, WK, WV, XF, WA, WB, W2 = 0, 1044, 1172, 1300, 1428, 1820, 1948, 2076
NPKB1 = 2112
P1A = 1428           # chunk A = XS+WQ+WK+WV, chunk B = rest
# pkb2 (bf16, 128 x 2436): selectors, mask, bias rows, Wo
B3, MT, MV, DEN36, ONESV = 0, 44, 80, 472, 508
BA1R, BOR, WOB, E9 = 900, 1028, 1156, 1284
NPKB2 = 2436
P2H = 1156           # head chunk, tail = WOB+esel
# pkf (f32, 128 x 4): biases (pre-scaled for the tanh-form activations)
BA2, BA1T, BA1H, BO = 0, 1, 2, 3
NPKF = 4

NWARM = 10           # PE warm-up matmuls (set pe_busy_start early)
NCHAIN = 15          # DVE copy chain gating the blocker matmuls
NKV = B * RH * WP    # 1044


def _emit_body(nc, tc, v):
    work, robpool, ebsbpool = v["work"], v["robpool"], v["ebsbpool"]
    pkb1, pkb2, pkf, out_d = v["pkb1"], v["pkb2"], v["pkf"], v["out_d"]

    qsb = work.tile([C, B, RQ, W], BF16, tag="qsb")
    ksb = work.tile([C, B, RH, WP], BF16, tag="ksb")
    vsb = work.tile([C, B, RH, WP], BF16, tag="vsb")
    pp = work.tile([C, 9, B, RQ, W], BF16, tag="pp")
    a1s = work.tile([C, B, RQ, 28], BF16, tag="a1s")
    osb = work.tile([C, B, RQ, W], BF16, tag="osb")
    th = work.tile([C, B, RQ, 28], BF16, tag="th")
    apad = work.tile([S36, B, RQ, 30], BF16, tag="apad")
    u_t = work.tile([S36, B, RQ, 28, 2], BF16, tag="ut")
    t1 = work.tile([S36, B, NQ], BF16, tag="t1")

    xs4 = pkb1[:, XS:XS + 1044].rearrange("p (b r c) -> p b r c", b=B, r=RH)
    xsf = pkb1[:, XS:XS + 1044]
    xf4 = pkb1[:, XF:XF + 392].rearrange("p (b r c) -> p b r c", b=B, r=RQ)
    wob = pkb2[:, WOB:WOB + C]
    ones_v = pkb2[0:1, ONESV:ONESV + NQ]
    ba1h_row = pkb2[0:1, BA1R:BA1R + C]
    bo_row = pkb2[0:1, BOR:BOR + C]
    ba2_v = pkf[0:S36, BA2:BA2 + 1]
    mt_v = pkb2[0:4, MT:MT + S36]
    mv_v = pkb2[0:4, MV:MV + NQ]
    w2v = pkb1[:, W2:W2 + S36]

    def kshift(base, o):
        di, lo = o // 3 - 1, o % 3
        return base[:, :, 1 + di:8 + di, lo:lo + W]

    def _adddim(apx, stride, size):
        dims = [list(d) for d in apx.ap]
        return bass.AP(apx.tensor, apx.offset,
                       [dims[0], [stride, size]] + dims[1:])

    def ppmul(eng, b, r):
        # one op for offsets 3r..3r+2 (same row band, col strides 1)
        o = 3 * r
        di = r - 1
        in1 = _adddim(ksb[:, b, 1 + di:8 + di, 0:W], 1, 3)
        in0 = _adddim(qsb[:, b, :, :], 0, 3)
        eng.tensor_mul(pp[:, o:o + 3, b, :, :], in0, in1)

    def dmm(d, b, o, start):
        nc.tensor.matmul(d[:, :], pkb2[:, B3 + 8 - o:B3 + 44 - o],
                         pp[:, o, b, :, :], start=start, stop=False)

    # ---- phase 1 ----
    # dotsp opens first so the dots/den/a2 banks never wait on the
    # projection pool's release (pool frees are scope-, not bank-grained).
    with tc.tile_pool(name="dotsp", bufs=2, space="PSUM") as dotsp:
        dots0 = dotsp.tile([S36, NQ], F32, tag="dots")
        a2ps = dotsp.tile([S36, NQ], F32, tag="dots")

        with tc.tile_pool(name="psA", bufs=1, space="PSUM") as psA:
            qps = psA.tile([C, B, 512], F32, tag="qps")
            a1ps = psA.tile([C, B * RQ * 28], F32, tag="a1ps")
            kvps = psA.tile([C, NKV], F32, tag="kvps")

            for lo in (0, 512, 1024):
                hi = min(lo + 512, NKV)
                nc.tensor.matmul(kvps[:, lo:hi], pkb1[:, WK:WK + C],
                                 xsf[:, lo:hi], start=True, stop=True)
            ksbf = ksb[:, :, :, :].rearrange("p b r c -> p (b r c)")
            nc.scalar.copy(out=ksbf[:, :], in_=kvps[:, :])
            for b in range(B):
                nc.tensor.matmul(qps[:, b, 0:NQ], pkb1[:, WQ:WQ + C],
                                 xs4[:, b, 1:8, 1:57], start=True, stop=True)
            nc.vector.tensor_copy(out=qsb[:, 1, :, :], in_=qps[:, 1, 0:NQ])
            nc.vector.tensor_copy(out=qsb[:, 0, :, :], in_=qps[:, 0, 0:NQ])
            # a1ps holds 0.5*(Wa1@cat + ba1): weights pre-halved on host,
            # bias accumulated via a rank-1 ones-row matmul
            nc.tensor.matmul(a1ps[:, :], pkb1[:, WA:WA + C],
                             xs4[:, :, 1:8, 1:29], start=True, stop=False)
            nc.tensor.matmul(a1ps[:, :], pkb1[:, WB:WB + C],
                             xf4[:, :, :, :], start=False, stop=False)
            nc.tensor.matmul(a1ps[:, :], ba1h_row, ones_v,
                             start=False, stop=True)
            # gelu(z) = (1+tanh(1.702*a1ps))*a1ps with a1ps = z/2
            nc.scalar.activation(th[:, :, :, :], a1ps[:, :], AF.Tanh,
                                 bias=0.0, scale=1.702)
            # V projection reuses the K banks once the ksb evacs land
            for lo in (0, 512, 1024):
                hi = min(lo + 512, NKV)
                nc.tensor.matmul(kvps[:, lo:hi], pkb1[:, WV:WV + C],
                                 xsf[:, lo:hi], start=True, stop=True)
            ppmul(nc.vector, 0, 0)
            nc.vector.scalar_tensor_tensor(
                out=a1s[:, :, :, :].rearrange("p b r c -> p (b r c)"),
                in0=th[:, :, :, :].rearrange("p b r c -> p (b r c)"),
                scalar=1.0, in1=a1ps[:, :], op0=ALU.add, op1=ALU.mult)
            for b in range(B):
                nc.tensor.matmul(a2ps[:, b * 196:(b + 1) * 196], w2v,
                                 a1s[:, b, :, :], start=True, stop=True)
            # apad holds tanh(a2/2); sigmoid folds into the resize consts
            nc.scalar.activation(apad[:, :, :, 1:29], a2ps[:, :],
                                 AF.Tanh, bias=ba2_v, scale=0.5)
            ppmul(nc.gpsimd, 1, 0)     # Pool: batch-1 offsets 0..5
            ppmul(nc.vector, 0, 1)
            for o in (0, 1, 2, 3, 4, 5):
                dmm(dots0, 0, o, o == 0)
            ppmul(nc.gpsimd, 1, 1)
            ppmul(nc.vector, 0, 2)
            for o in (6, 7, 8):
                dmm(dots0, 0, o, False)
            nc.tensor.matmul(dots0[:, :], mt_v, mv_v, start=False, stop=True)
            # resize pads + 28->56 bilinear; gate = 1.25 + 0.25*resize(tanh)
            nc.vector.tensor_copy(out=apad[:, :, :, 0:1],
                                  in_=apad[:, :, :, 1:2])
            nc.vector.tensor_copy(out=apad[:, :, :, 29:30],
                                  in_=apad[:, :, :, 28:29])
            nc.vector.scalar_tensor_tensor(
                out=u_t[:, :, :, :, 0], in0=apad[:, :, :, 1:29], scalar=3.0,
                in1=apad[:, :, :, 0:28], op0=ALU.mult, op1=ALU.add)
            nc.vector.scalar_tensor_tensor(
                out=u_t[:, :, :, :, 1], in0=apad[:, :, :, 1:29], scalar=3.0,
                in1=apad[:, :, :, 2:30], op0=ALU.mult, op1=ALU.add)
            nc.vector.tensor_scalar(
                out=t1[:, :, :].rearrange("p b q -> p (b q)"),
                in0=u_t[:, :, :, :, :].rearrange("p b r c t -> p (b r c t)"),
                scalar1=0.03125, scalar2=1.25, op0=ALU.mult, op1=ALU.add)
            vsbf = vsb[:, :, :, :].rearrange("p b r c -> p (b r c)")
            nc.scalar.copy(out=vsbf[:, :], in_=kvps[:, :])

        def softmax(b, dots):
            e_t = work.tile([S36, NQ], BF16, tag=f"e{b}", name=f"e_t{b}")
            nc.scalar.activation(e_t[:, :], dots[:, :], AF.Exp, scale=SCALE)
            den = dotsp.tile([S36, NQ], F32, tag="dots")
            nc.tensor.matmul(den[:, :], pkb2[0:S36, DEN36:DEN36 + S36],
                             e_t[:, :], start=True, stop=True)
            et1 = work.tile([S36, NQ], BF16, tag=f"s{b}", name=f"et1{b}")
            nc.vector.tensor_mul(et1[:, :], e_t[:, :], t1[:, b, :])
            r_t = work.tile([S36, NQ], F32, tag=f"r{b}", name=f"r_t{b}")
            nc.vector.reciprocal_approx_fast(r_t[:, :], den[:, :])
            e2 = work.tile([S36, NQ], BF16, tag=f"e2{b}", name=f"e2{b}")
            nc.vector.tensor_mul(e2[:, :], et1[:, :], r_t[:, :])
            return e2

        e20 = softmax(0, dots0)
        ppmul(nc.vector, 1, 2)     # DVE: batch-1 offsets 6..8

        with tc.tile_pool(name="psB", bufs=1, space="PSUM") as psB, \
             tc.tile_pool(name="ebpool", bufs=2, space="PSUM") as ebpool:
            outp = psB.tile([C, B, 512], F32, tag="outp")
            dots1_box = [None]

            def vpair(b, p):
                # V slices for offsets (2p, 2p+1) as one 4-D AP
                o0, o1 = 2 * p, 2 * p + 1
                d0, l0 = o0 // 3 - 1, o0 % 3
                d1, l1 = o1 // 3 - 1, o1 % 3
                js = (d1 - d0) * WP + (l1 - l0)
                return _adddim(vsb[:, b, 1 + d0:8 + d0, l0:l0 + W], js, 2)

            def d8_stage(b, e2):
                # direct-from-PSUM offset 8 + bias open the group
                ebp8 = ebpool.tile([C, 2, 512], F32, tag="ebp")
                nc.tensor.matmul(ebp8[:, 0, 0:NQ],
                                 pkb2[0:S36, E9 + C * 8:E9 + C * 9],
                                 e2[:, :], start=True, stop=True)
                nc.tensor.matmul(outp[:, b, 0:NQ], bo_row, ones_v,
                                 start=True, stop=False)
                rob8 = robpool.tile([C, 2, RQ, W], BF16, tag="rob")
                nc.vector.tensor_mul(rob8[:, 0, :, :], ebp8[:, 0, 0:NQ],
                                     kshift(vsb, 8)[:, b])
                nc.tensor.matmul(outp[:, b, 0:NQ], wob,
                                 rob8[:, 0, :, :].rearrange(
                                     "p r c -> p (r c)"),
                                 start=False, stop=False)

            def pair_front(b, e2, p, rob_eng=None):
                ebp2 = ebpool.tile([C, 2, 512], F32, tag="ebp")
                for j in (0, 1):
                    o = 2 * p + j
                    nc.tensor.matmul(
                        ebp2[:, j, 0:NQ],
                        pkb2[0:S36, E9 + C * o:E9 + C * (o + 1)],
                        e2[:, :], start=True, stop=True)
                ebsb2 = ebsbpool.tile([C, 2, NQ], BF16, tag="ebsb")
                nc.scalar.copy(out=ebsb2[:, :, :], in_=ebp2[:, :, 0:NQ])
                rob2 = robpool.tile([C, 2, RQ, W], BF16, tag="rob")
                (rob_eng or nc.vector).tensor_mul(
                    rob2[:, :, :, :],
                    ebsb2[:, :, :].rearrange("p j (r c) -> p j r c", r=RQ),
                    vpair(b, p))
                return rob2

            def pair_back(b, p, rob2, stop=False):
                for j in (0, 1):
                    nc.tensor.matmul(outp[:, b, 0:NQ], wob,
                                     rob2[:, j, :, :].rearrange(
                                         "p r c -> p (r c)"),
                                     start=False, stop=(stop and j == 1))

            # batch-0 and batch-1 pair streams interleaved so ACT's pair
            # evacs run back-to-back; batch-1 joins once its softmax lands.
            d8_stage(0, e20)
            d = dotsp.tile([S36, NQ], F32, tag="dots")
            for o in (0, 1, 2, 3, 4):
                dmm(d, 1, o, o == 0)
            r00 = pair_front(0, e20, 0)
            for o in (5, 6, 7, 8):
                dmm(d, 1, o, False)
            nc.tensor.matmul(d[:, :], mt_v, mv_v, start=False, stop=True)
            r01 = pair_front(0, e20, 1)
            pair_back(0, 0, r00)
            e21 = softmax(1, d)
            r02 = pair_front(0, e20, 2)
            pair_back(0, 1, r01)
            d8_stage(1, e21)
            r03 = pair_front(0, e20, 3)
            pair_back(0, 2, r02)
            r10 = pair_front(1, e21, 0)
            pair_back(0, 3, r03, stop=True)
            nc.vector.tensor_copy(out=osb[:, 0, :, :], in_=outp[:, 0, 0:NQ])
            nc.sync.dma_start(out=out_d[:, 0, :, :], in_=osb[:, 0, :, :])
            r11 = pair_front(1, e21, 1, rob_eng=nc.gpsimd)
            pair_back(1, 0, r10)
            r12 = pair_front(1, e21, 2)
            pair_back(1, 1, r11)
            r13 = pair_front(1, e21, 3)
            pair_back(1, 2, r12)
            pair_back(1, 3, r13, stop=True)
            nc.vector.tensor_copy(out=osb[:, 1, :, :], in_=outp[:, 1, 0:NQ])
            nc.sync.dma_start(out=out_d[:, 1, :, :], in_=osb[:, 1, :, :])


def build(repeat=1):
    nc = bacc.Bacc(num_devices=NCORES, debug=False)
    pkb1_d = nc.dram_tensor("pkb1", (C, NPKB1), BF16, kind="ExternalInput")
    pkb2_d = nc.dram_tensor("pkb2", (C, NPKB2), BF16, kind="ExternalInput")
    pkf_d = nc.dram_tensor("pkf", (C, NPKF), F32, kind="ExternalInput")
    out_d = nc.dram_tensor("out", (C, B, RQ, W), BF16, kind="ExternalOutput")

    with tile.TileContext(nc) as tc:
        with tc.tile_pool(name="consts", bufs=1) as consts, \
             tc.tile_pool(name="warm", bufs=1) as warm, \
             tc.tile_pool(name="work", bufs=1) as work, \
             tc.tile_pool(name="robp", bufs=3) as robpool, \
             tc.tile_pool(name="ebsbp", bufs=3) as ebsbpool:
            pkb1_t = consts.tile([C, NPKB1], BF16, tag="pkb1")
            pkb2_t = consts.tile([C, NPKB2], BF16, tag="pkb2")
            pkf_t = consts.tile([C, NPKF], F32, tag="pkf")
            # input DMAs, criticality-ordered; SP and ACT queues in parallel
            nc.sync.dma_start(out=pkb1_t[:, 0:P1A], in_=pkb1_d[:, 0:P1A])
            nc.scalar.dma_start(out=pkb2_t[:, 0:P2H], in_=pkb2_d[:, 0:P2H])
            nc.scalar.dma_start(out=pkf_t[:, :], in_=pkf_d[:, :])
            nc.sync.dma_start(out=pkb1_t[:, P1A:NPKB1],
                              in_=pkb1_d[:, P1A:NPKB1])
            nc.sync.dma_start(out=pkb2_t[:, P2H:NPKB2],
                              in_=pkb2_d[:, P2H:NPKB2])

            # PE warm-up: set pe_busy_start early, then block the PE wait
            # queue until the p-state ramp (3us) has elapsed so the real
            # matmuls are all costed at full clock.
            wt = warm.tile([C, 256], BF16, tag="wt")
            ch = [warm.tile([C, 256], BF16, tag=f"ch{i}", name=f"ch{i}")
                  for i in range(NCHAIN)]
            nc.gpsimd.memset(wt[:, :], 0.0)
            prev = wt
            for i in range(NCHAIN):
                nc.vector.tensor_copy(out=ch[i][:, :], in_=prev[:, :])
                prev = ch[i]
            with tc.tile_pool(name="wpool", bufs=1, space="PSUM") as wpool:
                wps = wpool.tile([C, 256], F32, tag="wps")
                # warmup j rides the copy chain so the PE never idles long
                # enough to reset its p-state ramp clock
                nc.tensor.matmul(wps[:, :], wt[:, 0:C], wt[:, :],
                                 start=True, stop=True)
                for i in range(NWARM):
                    src_t = ch[min(i, NCHAIN - 1)]
                    nc.tensor.matmul(wps[:, :], src_t[:, 0:C], src_t[:, :],
                                     start=True, stop=True)
                for i in range(2):
                    nc.tensor.matmul(wps[0:1, 0:64], prev[0:1, 0:1],
                                     prev[0:1, 0:64], start=True, stop=True)

            v = {"pkb1": pkb1_t, "pkb2": pkb2_t, "pkf": pkf_t,
                 "out_d": out_d, "work": work, "robpool": robpool,
                 "ebsbpool": ebsbpool}
            for i in range(repeat):
                _emit_body(nc, tc, v)
    nc.compile()
    return nc


def host_inputs(x, Wq, Wk, Wv, Wo, bo, Wa1, ba1, Wa2, ba2):
    import ml_dtypes
    BF = ml_dtypes.bfloat16
    pkb1 = np.zeros((C, NPKB1), BF)
    pkb1[:, WQ:WQ + C] = Wq.T
    pkb1[:, WK:WK + C] = Wk.T
    pkb1[:, WV:WV + C] = Wv.T
    pkb1[:, WA:WA + C] = 0.5 * Wa1[:, :C].T
    pkb1[:, WB:WB + C] = 0.5 * Wa1[:, C:].T
    pkb1[:, W2:W2 + S36] = Wa2[0][:, None]

    pkb2 = np.zeros((C, NPKB2), BF)
    # sliding ones-block selector (cols 64:108 of the legacy layout)
    base3 = np.zeros((C, 44), np.float32)
    for h in range(HEADS):
        base3[32 * h:32 * h + 32, 8 + 9 * h] = 1.0
    pkb2[:, B3:B3 + 44] = base3
    pkb2[:, WOB:WOB + C] = Wo.T
    esel = np.zeros((36, 9, C), np.float32)
    for o in range(9):
        for hh in range(HEADS):
            esel[9 * hh + o, o, 32 * hh:32 * hh + 32] = 1.0
    pkb2[0:36, E9:E9 + 9 * C] = esel.reshape(36, 9 * C)
    mt = np.zeros((4, S36), np.float32)
    for h in range(HEADS):
        for o in range(9):
            di, dj = o // 3 - 1, o % 3 - 1
            p = 9 * h + o
            mt[0, p] = 1.0 if dj == -1 else 0.0
            mt[1, p] = 1.0 if dj == 1 else 0.0
            mt[2, p] = 1.0 if di == -1 else 0.0
            mt[3, p] = 1.0 if di == 1 else 0.0
    pkb2[0:4, MT:MT + S36] = mt
    pkb2[0, ONESV:ONESV + NQ] = 1.0    # ones row for the bias matmuls
    pkb2[0, BA1R:BA1R + C] = 0.5 * ba1  # bias row (pre-halved like Wa1)
    pkb2[0, BOR:BOR + C] = bo          # output bias row

    den36 = np.zeros((S36, S36), np.float32)
    for h in range(HEADS):
        den36[9 * h:9 * h + 9, 9 * h:9 * h + 9] = 1.0
    pkb2[0:S36, DEN36:DEN36 + S36] = den36

    pkf = np.zeros((C, NPKF), np.float32)
    pkf[0:S36, BA2] = 0.5 * float(ba2[0])
    pkf[:, BO] = bo

    in_maps = []
    for c in range(NCORES):
        r0 = 7 * c
        rows = np.clip(np.arange(r0 - 1, r0 + 8), 0, 55)
        cols = np.clip(np.arange(-1, 57), 0, 55)
        xs = x[:, :, rows][:, :, :, cols].transpose(1, 0, 2, 3)
        xflip = x[:, :, r0:r0 + 7, :27:-1].transpose(1, 0, 2, 3)
        pkb1c = pkb1.copy()
        pkb1c[:, XS:XS + 1044] = xs.reshape(C, 1044)
        pkb1c[:, XF:XF + 392] = xflip.reshape(C, 392)
        mv = np.zeros((4, RQ, W), np.float32)
        cc, rr = np.arange(W), r0 + np.arange(RQ)
        mv[0, :, cc == 0] = -BIG
        mv[1, :, cc == 55] = -BIG
        mv[2, rr == 0, :] = -BIG
        mv[3, rr == 55, :] = -BIG
        pkb2c = pkb2.copy()
        pkb2c[0:4, MV:MV + NQ] = mv.reshape(4, NQ)
        in_maps.append({"pkb1": pkb1c, "pkb2": pkb2c, "pkf": pkf})
    return in_maps


_NC = None


def _get_nc():
    global _NC
    if _NC is None:
        _NC = build()
    return _NC


def kernel(**inputs):
    args = {k: np.asarray(v, np.float32) for k, v in inputs.items()}
    nc = _get_nc()
    in_maps = host_inputs(
        args["x"], args["Wq"], args["Wk"], args["Wv"], args["Wo"],
        args["bo"], args["Wa1"], args["ba1"], args["Wa2"], args["ba2"])
    res = run_bass_kernel_spmd(nc, in_maps, core_ids=list(range(NCORES)))
    y = np.empty((B, C, H, W), np.float32)
    for c in range(NCORES):
        y[:, :, 7 * c:7 * c + 7, :] = np.asarray(
            res.results[c]["out"], np.float32).transpose(1, 0, 2, 3)
    return y
